# revision 1
# baseline (speedup 1.0000x reference)
"""Multi-head attention with relative position bias on 8 trn2 NeuronCores.

Sharding: data-parallel on batch (2) x tensor-parallel on heads (16 -> 4 per
core).  Core c handles batch c//4, heads 4*(c%4) .. 4*(c%4)+3.  Each core
computes its 4 heads' attention and a partial output projection (contraction
over its 256 columns of the head-concat dim); the host sums the 4 partials per
batch (divided by the 128x fp8 scale) and adds b_out.

Device-side design (per core):
  - x^T [1024, 2048] host-transposed fp32r; W_q/W_k/W_v^T fp32r.  QKV
    projections run on the fp32r path (1 col/cycle), q/k stored bf16,
    v stored fp8e4 (with a ones column for the softmax denominator).
  - scores computed transposed: S^T[nk, nq] = kT-block @ qT (contraction over
    dh=64 on partitions), bf16 inputs, fp32 PSUM.  1/8 scale folded into W_k
    on the host.
  - rel-pos bias is Toeplitz by 128x128 tile: 17 distinct tiles per head
    (|delta| <= 8) + 2 saturated edge constants.  All bias tiles are DMA'd
    ONCE into SBUF (resident), not per query-chunk.
  - mid/mixed key-tiles: DVE adds bias tiles into pt (bf16); Act exps the
    contiguous mid-run in big slabs, writing pt8 (fp8e4) directly.
    Pure-edge key-tiles (|kt-qi| >= 9 for all 4 q-blocks of the chunk) skip
    DVE entirely: Act exps them straight from PSUM with the saturated bias
    constant as a per-partition bias operand.
  - P@V runs in fp8e4 DoubleRow perf mode: each matmul consumes TWO key
    tiles (lhsT [128, 2, 65] v-pairs, rhs [128, 2, 512] pt8-pairs) at 0.5
    cycles/col -- 4x fewer PE cycles than the bf16 version.
  - softmax denominator: l row = pv[64] (ones column).  1/l via a single
    custom-DVE reciprocal_approx_fast on the [1, 512] PSUM row, then a K=1
    fp32r matmul against a constant 8.0 row broadcasts 8/l to 64 partitions
    (the 8x is fp8-range scaling for attn values, folded into the host-side
    1/128).  One DVE multiply writes attn8 (fp8e4).
  - output projection in fp8e4 DoubleRow: each matmul consumes TWO heads
    (lhsT [64, 2, 128] attn8 head-pairs, rhs [64, 2, 512] wo8 head-pairs,
    wo8 host-scaled by 16).  Partial output returned at 128x scale in bf16.
  - ONE shared PSUM pool for the whole kernel (tags s0-s3 / pv0-pv1 / bc,
    7 banks) -- no pool-transition release-wait bursts, and the scheduler
    can overlap the QKV / attention / out-projection phases freely.
  - fp32r matmults lower to a single struct with ONE sync-wait slot: a
    write-NoOp "gate" precedes every accumulation group to absorb the PSUM
    slot-release waits, and a post-schedule pass (_fix_sync_waits) elides
    redundant waits and moves any residual excess onto the gate.
"""

import sys

import numpy as np

if "/opt/trn_rl_repo" not in sys.path:
    sys.path.insert(0, "/opt/trn_rl_repo")

import ml_dtypes

import concourse.bass as bass
import concourse.mybir as mybir
import concourse.tile as tile
from concourse.bass_utils import run_bass_kernel_spmd

F32 = mybir.dt.float32
F32R = mybir.dt.float32r
BF16 = mybir.dt.bfloat16
FP8 = mybir.dt.float8e4
EXP = mybir.ActivationFunctionType.Exp
DR = mybir.MatmulPerfMode.DoubleRow

N = 2048  # sequence length
DIM = 1024  # model dim
HL = 4  # local heads per core
DH = 64  # head dim
NKT = N // 128  # 16 key tiles
QC = 512  # query-chunk width
NQC = N // QC  # 4 query chunks
NDT = DIM // 128  # 8 contraction tiles for the projections

_PROGRAM = None
LAST_RESULTS = None  # BassKernelResults of the most recent run (for test.py)


def _pe_gate(tc, outs):
    """PE NoOp that 'writes' the given psum APs: it becomes the tile's first
    writer, so the PSUM slot-release waits land on the NoOp instead of the
    following fp32r matmul (which has a single sync-wait slot)."""
    nc = tc.nc
    inst = mybir.InstNoOp(
        name=nc.get_next_instruction_name(),
        ins=[],
        outs=[nc.tensor.lower_ap(ap) for ap in outs],
    )
    inst.bass_nofuse = True
    return nc.tensor.add_instruction(inst)


def _gate_dep(a, b):
    bass._add_dep_helper(a.ins, b.ins, sync=False, reason="f32r 1-wait gate")


def _segments(kt, c):
    """Bias treatment for score chunk (kt, c) split into runs over the 4
    query 128-blocks: ('mid', i0) -> tensor_add with bias[i0 : i0+len],
    ('edge', side) -> tensor_scalar_add with edge constant (0=lo, 1=hi)."""
    kinds = []
    for j in range(QC // 128):
        qi = (QC // 128) * c + j
        delta = kt - qi
        if delta >= 9:
            kinds.append(("edge", 1))
        elif delta <= -9:
            kinds.append(("edge", 0))
        else:
            kinds.append(("mid", 8 - delta))
    segs = []
    j = 0
    while j < len(kinds):
        j1 = j + 1
        while j1 < len(kinds) and kinds[j1][0] == kinds[j][0] and (
            kinds[j][0] == "edge" and kinds[j1][1] == kinds[j][1]
            or kinds[j][0] == "mid"
        ):
            j1 += 1
        segs.append((j, j1, kinds[j][0], kinds[j][1]))
        j = j1
    return segs


def _emit(tc, xT, wqT, wkT, wvT, woT, biasT, bias_edge, rec_dram, out_p):
    nc = tc.nc

    with (
        tc.tile_pool(name="persist", bufs=1) as persist,
        tc.tile_pool(name="pt", bufs=2) as ptp,
        tc.tile_pool(name="small", bufs=2) as smp,
        tc.tile_pool(name="ostp", bufs=2) as ostp,
        tc.tile_pool(name="ps", bufs=1, space="PSUM") as ps,
    ):
        # ---- constants + persistent tensors -------------------------------
        edge_sb = persist.tile([128, HL, 2], F32)
        edge_bcast = bass.AP(
            tensor=bias_edge.tensor,
            offset=bias_edge.offset,
            ap=[[0, 128]] + list(bias_edge.ap),
        )
        nc.gpsimd.dma_start(out=edge_sb, in_=edge_bcast)

        attn_sb = persist.tile([64, HL, N], BF16)  # normalized attn outputs
        wo_sb = persist.tile([64, HL, DIM], BF16)
        bias_all = persist.tile([128, HL, 17, 128], BF16)

        q_sb = persist.tile([128, 2, N], BF16)  # [2 heads x dh, pair, n]
        k_sb = persist.tile([128, 2, N], BF16)
        v_sb = persist.tile([128, NKT, HL, DH + 1], BF16)  # + ones column
        nc.vector.memset(v_sb[:, :, :, DH : DH + 1], 1.0)

        x_sb = persist.tile([128, NDT, N], F32R)
        wq_sb = persist.tile([128, NDT, 256], F32R)
        wk_sb = persist.tile([128, NDT, 256], F32R)
        wv_sb = persist.tile([128, NDT, 256], F32R)
        # startup loads spread across the three DMA initiators (scalar/
        # sync hwdge + gpsimd swdge) so the x/weight/bias transfers run on
        # parallel DMA queues instead of serializing behind one ring
        nc.scalar.dma_start(out=wk_sb, in_=wkT.rearrange("(t p) e -> p t e", p=128))
        nc.scalar.dma_start(out=wq_sb, in_=wqT.rearrange("(t p) e -> p t e", p=128))
        nc.scalar.dma_start(out=wv_sb, in_=wvT.rearrange("(t p) e -> p t e", p=128))
        # bias is host-transposed to partition-major so this DMA reads 128
        # contiguous 17KB rows instead of 8704 x 256B descriptors
        nc.scalar.dma_start(out=bias_all, in_=biasT)
        nc.scalar.dma_start(out=wo_sb, in_=woT.rearrange("(h p) e -> p h e", p=64))
        # chunk-major x load so the first q/k projections can start early;
        # alternate dt-tiles between the sync and gpsimd queues
        for c in range(NQC):
            for dt in range(NDT):
                eng = nc.sync if dt % 2 == 0 else nc.gpsimd
                eng.dma_start(
                    out=x_sb[:, dt, c * QC : (c + 1) * QC],
                    in_=xT[dt * 128 : (dt + 1) * 128, c * QC : (c + 1) * QC],
                )

        # ---- Phase A: QKV projections (fp32r) -----------------------------
        def _proj_qk(wsb, osb, ep):
            for c in range(NQC):
                p = ps.tile([128, QC], F32, tag=f"s{c % 3}")
                gate = _pe_gate(tc, [p[:, :]])
                for dt in range(NDT):
                    mm = nc.tensor.matmul(
                        p,
                        lhsT=wsb[:, dt, ep * 128 : (ep + 1) * 128],
                        rhs=x_sb[:, dt, c * QC : (c + 1) * QC],
                        start=(dt == 0),
                        stop=(dt == NDT - 1),
                    )
                    _gate_dep(mm, gate)
                nc.any.tensor_copy(osb[:, ep, c * QC : (c + 1) * QC], p)

        _proj_qk(wk_sb, k_sb, 0)
        _proj_qk(wq_sb, q_sb, 0)
        for kt in range(NKT):
            p = ps.tile([128, 256], F32, tag=f"pv{kt % 2}")
            gate = _pe_gate(tc, [p[:, :]])
            for dt in range(NDT):
                mm = nc.tensor.matmul(
                    p,
                    lhsT=x_sb[:, dt, kt * 128 : (kt + 1) * 128],
                    rhs=wv_sb[:, dt, :],
                    start=(dt == 0),
                    stop=(dt == NDT - 1),
                )
                _gate_dep(mm, gate)
            nc.any.tensor_copy(v_sb[:, kt, :, 0:DH], p)
        _proj_qk(wk_sb, k_sb, 1)
        _proj_qk(wq_sb, q_sb, 1)

        # ---- Phase B: attention -------------------------------------------
        for c in range(NQC):
            for h in range(HL):
                hp, hr = divmod(h, 2)
                qrow = hr * 64
                pt = ptp.tile([128, NKT, QC], BF16, tag="pt")
                mid_kts = []
                for kt in range(NKT):
                    p = ps.tile([128, QC], F32, tag=f"s{kt % 6}")
                    gate = _pe_gate(tc, [p[:, :]])
                    mm = nc.tensor.matmul(
                        p,
                        lhsT=k_sb[qrow : qrow + 64, hp, kt * 128 : (kt + 1) * 128],
                        rhs=q_sb[qrow : qrow + 64, hp, c * QC : (c + 1) * QC],
                        start=True,
                        stop=True,
                    )
                    _gate_dep(mm, gate)
                    segs = _segments(kt, c)
                    if len(segs) == 1 and segs[0][2] == "edge":
                        # fully saturated tile: exp straight from PSUM with
                        # the constant bias; no DVE pass
                        side = segs[0][3]
                        nc.scalar.activation(
                            pt[:, kt, :], p, EXP,
                            bias=edge_sb[:, h, side : side + 1],
                        )
                        continue
                    mid_kts.append(kt)
                    for j0, j1, kind, idx in segs:
                        dst = pt[:, kt, j0 * 128 : j1 * 128]
                        src = p[:, j0 * 128 : j1 * 128]
                        if kind == "mid":
                            nc.any.tensor_add(
                                dst, src, bias_all[:, h, idx : idx + (j1 - j0), :]
                            )
                        else:
                            nc.any.tensor_scalar_add(
                                dst, src, edge_sb[:, h, idx : idx + 1]
                            )
                # mid kts form one contiguous run; exp it in <=4-kt slabs
                k0, k1 = mid_kts[0], mid_kts[-1] + 1
                for s0 in range(k0, k1, 4):
                    s1 = min(s0 + 4, k1)
                    nc.scalar.activation(
                        pt[:, s0:s1, :], pt[:, s0:s1, :], EXP
                    )

                pv = ps.tile([DH + 1, QC], F32, tag=f"pv{(h * NQC + c) % 2}")
                gate = _pe_gate(tc, [pv[:, :]])
                for kt in range(NKT):
                    mm = nc.tensor.matmul(
                        pv,
                        lhsT=v_sb[:, kt, h, :],
                        rhs=pt[:, kt, :],
                        start=(kt == 0),
                        stop=(kt == NKT - 1),
                    )
                    _gate_dep(mm, gate)
                # softmax denominator: 1/l = exp(-ln l) on the Act LUTs (the
                # DVE reciprocal is ~6 passes/element; walrus rejects the
                # custom-DVE approx).  The fp8-range x8 for attn values is
                # folded into wv on the host (the ones column keeps l
                # unscaled).
                rec = smp.tile([65, QC], F32, tag=f"rec{(h * NQC + c) % 2}")
                nc.scalar.activation(
                    rec[64:65, :], pv[DH : DH + 1, :],
                    mybir.ActivationFunctionType.Ln,
                )
                nc.scalar.activation(
                    rec[64:65, :], rec[64:65, :], EXP, scale=-1.0
                )
                # partition-broadcast via a DRAM roundtrip (SBUF APs cannot
                # have partition stride 0; DRAM source APs can)
                idx = c * HL + h
                nc.gpsimd.dma_start(
                    out=rec_dram[idx : idx + 1, :], in_=rec[64:65, :]
                )
                bc_sb = smp.tile([64, QC], F32, tag=f"bc{(h * NQC + c) % 2}")
                nc.gpsimd.dma_start(
                    out=bc_sb,
                    in_=bass.AP(
                        tensor=rec_dram.tensor,
                        offset=idx * QC,
                        ap=[[0, 64], [1, QC]],
                    ),
                )
                nc.any.tensor_mul(
                    attn_sb[:, h, c * QC : (c + 1) * QC], pv[0:DH, :], bc_sb
                )

        # ---- Phase C: output projection (fp8 DoubleRow, 128x scale) -------
        for qi in range(N // 128):
            ost = ostp.tile([128, DIM], BF16, tag="ost")
            for nch in range(2):
                p = ps.tile([128, 512], F32, tag=f"s{(qi * 2 + nch) % 6}")
                gate = _pe_gate(tc, [p[:, :]])
                for h in range(HL):
                    mm = nc.tensor.matmul(
                        p,
                        lhsT=attn_sb[:, h, qi * 128 : (qi + 1) * 128],
                        rhs=wo_sb[:, h, nch * 512 : (nch + 1) * 512],
                        start=(h == 0),
                        stop=(h == HL - 1),
                    )
                    _gate_dep(mm, gate)
                nc.any.tensor_copy(ost[:, nch * 512 : (nch + 1) * 512], p)
            nc.sync.dma_start(out=out_p[qi * 128 : (qi + 1) * 128, :], in_=ost)


def _fix_sync_waits(nc):
    """Post-schedule wait hygiene for walrus's per-struct sync-wait limits.

    1. Elide waits already implied by an earlier wait on the same engine
       (sem-ge is monotone and engines execute their instructions in order).
    2. For instructions still over their struct's wait capacity, INSERT
       NoOp wait-carriers on the same engine directly before them (strictly
       more conservative: the waits execute earlier in the same engine
       order).
    """
    import re

    _elidable = re.compile(r"^(DMASW|DMAHW|PE|DVE|Activation|Pool|SP)")
    # only instruction types whose sync_info round-trips cleanly may be
    # touched; anything else (raw-ISA customs, barriers, drains, branches)
    # is left intact and clears the elision state conservatively
    _touchable = (
        mybir.InstMatmult,
        mybir.InstNoOp,
        mybir.InstTensorTensor,
        mybir.InstTensorScalarPtr,
        mybir.InstActivation,
        mybir.InstTensorCopy,
        mybir.InstDMACopy,
        mybir.InstLdweights,
        mybir.InstMemset,
    )
    for f in nc.m.functions:
        for b in f.blocks:
            seen = {}
            for i in b.instructions:
                si = i.sync_info
                if si is None or not si.on_wait:
                    continue
                if not isinstance(i, _touchable):
                    seen.clear()
                    continue
                s = seen.setdefault(i.engine, {})
                kept = []
                for w in si.on_wait:
                    if (
                        w.wait_mode == "sem-ge-imm"
                        and _elidable.match(w.ant_name or "")
                        and s.get(w.id, -1) >= w.wait_value
                    ):
                        continue
                    kept.append(w)
                    if w.wait_mode == "sem-ge-imm" and _elidable.match(
                        w.ant_name or ""
                    ):
                        s[w.id] = w.wait_value
                if len(kept) != len(si.on_wait):
                    si.on_wait = kept

    # capacity per opcode (walrus setupSyncWait limits, found empirically:
    # Matmult fp32r=1, NoOp=1; others conservative)
    def cap_of(i):
        if isinstance(i, mybir.InstDrain):
            return 1  # spill the kernel-tail drain's wait pile onto NoOps
        if not isinstance(i, _touchable):
            return None
        return 1

    for f in nc.m.functions:
        for b in f.blocks:
            out = []
            for i in b.instructions:
                si = i.sync_info
                cap = cap_of(i)
                if si is not None and si.on_wait and cap is not None and len(
                    si.on_wait
                ) > cap:
                    waits = list(si.on_wait)
                    excess, keep = waits[:-cap], waits[-cap:]
                    while excess:
                        chunk, excess = excess[:1], excess[1:]
                        nop = mybir.InstNoOp(
                            name=nc.get_next_instruction_name(), ins=[], outs=[]
                        )
                        nop.engine = i.engine
                        nop.sync_info = mybir.SyncInfo(on_wait=chunk, on_update=[])
                        nop.bass_nofuse = True
                        out.append(nop)
                    si.on_wait = keep
                out.append(i)
            b.instructions = out


def build_program():
    global _PROGRAM
    if _PROGRAM is not None:
        return _PROGRAM
    nc = bass.Bass(trn_type="TRN2", target_bir_lowering=False, debug=False)
    xT = nc.dram_tensor("xT", [DIM, N], F32R, kind="ExternalInput").ap()
    wqT = nc.dram_tensor("wqT", [DIM, 256], F32R, kind="ExternalInput").ap()
    wkT = nc.dram_tensor("wkT", [DIM, 256], F32R, kind="ExternalInput").ap()
    wvT = nc.dram_tensor("wvT", [DIM, 256], F32R, kind="ExternalInput").ap()
    woT = nc.dram_tensor("woT", [256, DIM], BF16, kind="ExternalInput").ap()
    biasT = nc.dram_tensor("biasT", [128, HL, 17, 128], BF16, kind="ExternalInput").ap()
    bias_edge = nc.dram_tensor("bias_edge", [HL, 2], F32, kind="ExternalInput").ap()
    rec_dram = nc.dram_tensor("rec_scratch", [NQC * HL, QC], F32, kind="Internal").ap()
    out_p = nc.dram_tensor("out_p", [N, DIM], BF16, kind="ExternalOutput").ap()

    with tile.TileContext(nc) as tc:
        _emit(tc, xT, wqT, wkT, wvT, woT, biasT, bias_edge, rec_dram, out_p)
    _fix_sync_waits(nc)
    _PROGRAM = nc
    return nc


def _round_f32r(a):
    """Round fp32 to the PE's FP32R format (11 explicit mantissa bits,
    round-half-up at bit 12) - matches neuronxcc's static_cast_fp32_to_fp32r."""
    u = np.ascontiguousarray(a, np.float32).view(np.uint32)
    r = ((u.astype(np.uint64) + 0x800) & 0xFFFFF000).astype(np.uint32)
    return r.view(np.float32)


def make_in_maps(x, W_qkv, W_out, rel_emb):
    x = np.asarray(x, np.float32)
    W_qkv = np.asarray(W_qkv, np.float32)
    W_out = np.asarray(W_out, np.float32)
    rel_emb = np.asarray(rel_emb, np.float32)

    dd = np.arange(128)[:, None] - np.arange(128)[None, :]
    xTs = [_round_f32r(np.ascontiguousarray(x[b].T)) for b in range(x.shape[0])]
    in_maps = []
    for c in range(8):
        b, g = c // 4, c % 4
        sl = slice(g * 256, (g + 1) * 256)
        wq = W_qkv[g * 256 : (g + 1) * 256]
        wk = W_qkv[DIM + g * 256 : DIM + (g + 1) * 256] * np.float32(0.125)
        wv = W_qkv[2 * DIM + g * 256 : 2 * DIM + (g + 1) * 256]
        # -2.5 keeps exp(s + bias) under fp8e4m3's 240 max (scores are
        # ~N(0,1), max ~5.7 over the full tensor); it cancels in softmax
        SHIFT = np.float32(2.5)
        bT = np.empty((HL, 17, 128, 128), np.float32)
        for hl in range(HL):
            head = 4 * g + hl
            for i in range(17):
                idx = np.clip(128 * (8 - i) + dd, -1024, 1024) + 1024
                bT[hl, i] = rel_emb[idx, head] - SHIFT
        be = np.stack(
            [rel_emb[0, 4 * g : 4 * g + 4], rel_emb[2048, 4 * g : 4 * g + 4]], axis=1
        ) - SHIFT
        in_maps.append(
            {
                "xT": xTs[b],
                "wqT": _round_f32r(wq.T),
                "wkT": _round_f32r(wk.T),
                "wvT": _round_f32r(wv.T),
                "woT": np.ascontiguousarray(W_out[:, sl].T).astype(ml_dtypes.bfloat16),
                "biasT": np.ascontiguousarray(
                    bT.transpose(2, 0, 1, 3)
                ).astype(ml_dtypes.bfloat16),
                "bias_edge": np.ascontiguousarray(be),
            }
        )
    return in_maps


def combine_outputs(results, b_out):
    b_out = np.asarray(b_out, np.float32)
    out = np.empty((2, N, DIM), np.float32)
    for b in range(2):
        acc = results[4 * b]["out_p"].astype(np.float32)
        for g in range(1, 4):
            acc = acc + results[4 * b + g]["out_p"].astype(np.float32)
        out[b] = acc + b_out[None, :]
    return out


def kernel(x, W_qkv, W_out, b_out, rel_emb):
    global LAST_RESULTS
    nc = build_program()
    in_maps = make_in_maps(x, W_qkv, W_out, rel_emb)
    LAST_RESULTS = run_bass_kernel_spmd(nc, in_maps, list(range(8)))
    return combine_outputs(LAST_RESULTS.results, b_out)



# revision 12
# speedup vs baseline: 1.1937x; 1.1937x over previous
"""Multi-head attention with relative position bias on 8 trn2 NeuronCores.

Sharding: data-parallel on batch (2) x tensor-parallel on heads (16 -> 4 per
core).  Core c handles batch c//4, heads 4*(c%4) .. 4*(c%4)+3.  Each core
computes its 4 heads' attention and a partial output projection; the host sums
the 4 partials per batch and adds b_out.

Device-side design (per core), all bf16 on the PE (no fp8: e4m3's subnormal
floor costs ~2.3% rel err on softmax weights, over the 2e-2 gate):
  - x^T, W_q/W_k/W_v^T, W_out^T host-cast to bf16 (halves the startup DMA
    vs fp32r at the same 1 col/cycle PE rate; rel err 0.0079 in numpy sim).
  - QKV projections run dt-outer so one LdWeights serves the 4 matmuls of
    all query chunks (InstMatmult.ldweights=False on the followers).
  - scores computed transposed: S^T[k, q] = kT-tile @ qT, contraction dh=64
    on partitions; per k-tile ONE ldweights feeds both 512-col halves of a
    1024-wide query unit.  1/8 scale folded into W_k on the host.
  - rel-pos bias applied MULTIPLICATIVELY after exp: pt = exp(S) * exp(b).
    Act reads PSUM directly (no DVE pre-pass), writes bf16; DVE/Pool then
    multiply by host-precomputed exp(bias) Toeplitz tiles (17 per head,
    resident in SBUF), alternating engines by k-tile; saturated edge runs
    use tensor_scalar_mul, fully-saturated tiles fold ln(edge) into the
    Act bias operand.  bf16xbf16 multiplies run the DVE 2x path and the
    multiplicative form skips the baseline's bf16 rounding of S+b (which
    cost ~1.6% on large scores).
  - P@V per unit: 16 accumulating bf16 matmuls, lhsT = v-tile [128, 65]
    (ones column -> softmax denominator l in psum row 64).
  - softmax reciprocal BATCHED: l rows DMA to DRAM, gathered back as
    [128, 16] so ln/exp on Act use all 128 lanes (vs 1), rec returns via
    DRAM for the partition-broadcast multiply.
  - output projection with heads PAIRED to contraction 128: the normalize
    multiply writes odd heads partition-shifted into rows 64:128 of a
    [128, 2, n] attn tile (DVE supports shifted writes; HW-verified), so
    out-proj needs 2 accumulation steps instead of 4.
  - emission interleaves PE work at k-tile granularity so the PE never
    stalls on the 2-deep score-psum rotation while Act drains it: the
    first score unit interleaves v-projection chains, the second the ep1
    q/k projection passes, and every later unit the previous unit's P@V
    matmuls.  PSUM: 4 tags x 4KB (s0,s1 scores / pv0,pv1 P@V+projections).
  - _fix_sync_waits post-pass (unchanged from baseline) elides redundant
    semaphore waits and spills over-capacity wait lists onto NoOp carriers
    for walrus's per-struct sync-wait limits.
"""

import sys

import numpy as np

if "/opt/trn_rl_repo" not in sys.path:
    sys.path.insert(0, "/opt/trn_rl_repo")

import ml_dtypes

import concourse.bass as bass
import concourse.mybir as mybir
import concourse.tile as tile
from concourse.bass_utils import run_bass_kernel_spmd

F32 = mybir.dt.float32
BF16 = mybir.dt.bfloat16
EXP = mybir.ActivationFunctionType.Exp
LN = mybir.ActivationFunctionType.Ln

N = 2048  # sequence length
DIM = 1024  # model dim
HL = 4  # local heads per core
DH = 64  # head dim
NKT = N // 128  # 16 key tiles
CW = 1024  # query-unit width (2 units per head)
NDT = DIM // 128  # 8 contraction tiles for the projections

_PROGRAM = None
LAST_RESULTS = None  # BassKernelResults of the most recent run (for test.py)


def _segments(kt, u):
    """Bias treatment for score tile (kt, unit u) split into runs over the 8
    query 128-blocks: ('mid', i0) -> tensor_mul with expb[i0 : i0+len],
    ('edge', side) -> tensor_scalar_mul with edge constant (0=lo, 1=hi)."""
    kinds = []
    for j in range(CW // 128):
        qi = (CW // 128) * u + j
        delta = kt - qi
        if delta >= 9:
            kinds.append(("edge", 1))
        elif delta <= -9:
            kinds.append(("edge", 0))
        else:
            kinds.append(("mid", 8 - delta))
    segs = []
    j = 0
    while j < len(kinds):
        j1 = j + 1
        while j1 < len(kinds) and kinds[j1][0] == kinds[j][0] and (
            kinds[j][0] == "edge" and kinds[j1][1] == kinds[j][1]
            or kinds[j][0] == "mid"
        ):
            j1 += 1
        segs.append((j, j1, kinds[j][0], kinds[j][1]))
        j = j1
    return segs


def _emit(tc, xT, wqT, wkT, wvT, wo2T, expbT, edge_ln, edge_mul, l_dram,
          rec_dram, out_p):
    nc = tc.nc

    with (
        tc.tile_pool(name="persist", bufs=1) as persist,
        tc.tile_pool(name="pt", bufs=2) as ptp,
        tc.tile_pool(name="small", bufs=2) as smp,
        tc.tile_pool(name="ostp", bufs=2) as ostp,
        tc.tile_pool(name="ps", bufs=1, space="PSUM") as ps,
    ):
        # ---- constants + persistent tensors -------------------------------
        edgel_sb = persist.tile([128, HL, 2], F32)  # ln-domain (Act bias)
        edgem_sb = persist.tile([128, HL, 2], F32)  # exp-domain (scalar mul)
        for dst, src in ((edgel_sb, edge_ln), (edgem_sb, edge_mul)):
            bcast = bass.AP(
                tensor=src.tensor,
                offset=src.offset,
                ap=[[0, 128]] + list(src.ap),
            )
            nc.gpsimd.dma_start(out=dst, in_=bcast)

        attn2_sb = persist.tile([128, 2, N], BF16)  # paired heads, normalized
        wo2_sb = persist.tile([128, 2, DIM], BF16)
        expb_sb = persist.tile([128, HL, 17, 128], BF16)

        q_sb = persist.tile([128, 2, N], BF16)  # [2 heads x dh, ep, n]
        k_sb = persist.tile([128, 2, N], BF16)
        v_sb = persist.tile([128, NKT, HL, DH + 1], BF16)  # + ones column
        nc.vector.memset(v_sb[:, :, :, DH : DH + 1], 1.0)

        x_sb = persist.tile([128, NDT, N], BF16)
        wq_sb = persist.tile([128, NDT, 256], BF16)
        wk_sb = persist.tile([128, NDT, 256], BF16)
        wv_sb = persist.tile([128, NDT, 256], BF16)
        # weights + bias tables on the scalar hwdge queue; x split across
        # the sync and gpsimd queues dt-major so the dt-outer projections
        # can start as soon as the first dim-tiles land
        nc.scalar.dma_start(out=wk_sb, in_=wkT.rearrange("(t p) e -> p t e", p=128))
        nc.scalar.dma_start(out=wq_sb, in_=wqT.rearrange("(t p) e -> p t e", p=128))
        nc.scalar.dma_start(out=wv_sb, in_=wvT.rearrange("(t p) e -> p t e", p=128))
        nc.scalar.dma_start(out=expb_sb, in_=expbT)
        nc.scalar.dma_start(out=wo2_sb, in_=wo2T)
        for dt in range(NDT):
            eng = nc.sync if dt % 2 == 0 else nc.gpsimd
            eng.dma_start(
                out=x_sb[:, dt, :], in_=xT[dt * 128 : (dt + 1) * 128, :]
            )

        # ---- QKV projections (bf16, dt-outer, shared ldweights) -----------
        def proj4(wsb, osb, ep, tags):
            """All 4 query chunks of one 128-row output slab in a single
            dt-outer pass: 8 ldweights, 32 matmuls on two [128,1024] psums."""
            p0 = ps.tile([128, CW], F32, tag=tags[0])
            p1 = ps.tile([128, CW], F32, tag=tags[1])
            for dt in range(NDT):
                lhsT = wsb[:, dt, ep * 128 : (ep + 1) * 128]
                first = True
                for p, base in ((p0, 0), (p0, 512), (p1, 1024), (p1, 1536)):
                    mm = nc.tensor.matmul(
                        p[:, base % CW : base % CW + 512],
                        lhsT=lhsT,
                        rhs=x_sb[:, dt, base : base + 512],
                        start=(dt == 0),
                        stop=(dt == NDT - 1),
                    )
                    if not first:
                        mm.ins.ldweights = False
                    first = False
            nc.any.tensor_copy(osb[:, ep, 0:CW], p0)
            nc.any.tensor_copy(osb[:, ep, CW : 2 * CW], p1)

        def proj2(wsb, osb, ep, half, tag):
            """One chunk-pair of an ep1 slab on a single psum tag (used to
            interleave with score units without touching the s tags)."""
            p = ps.tile([128, CW], F32, tag=tag)
            for dt in range(NDT):
                lhsT = wsb[:, dt, ep * 128 : (ep + 1) * 128]
                mm0 = nc.tensor.matmul(
                    p[:, 0:512],
                    lhsT=lhsT,
                    rhs=x_sb[:, dt, half * CW : half * CW + 512],
                    start=(dt == 0),
                    stop=(dt == NDT - 1),
                )
                mm1 = nc.tensor.matmul(
                    p[:, 512:CW],
                    lhsT=lhsT,
                    rhs=x_sb[:, dt, half * CW + 512 : half * CW + CW],
                    start=(dt == 0),
                    stop=(dt == NDT - 1),
                )
                mm1.ins.ldweights = False
            nc.any.tensor_copy(osb[:, ep, half * CW : (half + 1) * CW], p)

        def v_chain(kt):
            p = ps.tile([128, 256], F32, tag=f"pv{kt % 2}")
            for dt in range(NDT):
                nc.tensor.matmul(
                    p,
                    lhsT=x_sb[:, dt, kt * 128 : (kt + 1) * 128],
                    rhs=wv_sb[:, dt, :],
                    start=(dt == 0),
                    stop=(dt == NDT - 1),
                )
            nc.any.tensor_copy(v_sb[:, kt, :, 0:DH], p)

        # ---- attention building blocks ------------------------------------
        pt_tiles = {}

        def sc_tile(h, u, kt):
            """One k-tile of unit (h, u): 2 score matmuls (shared ldweights),
            exp from psum, multiplicative bias."""
            hp, hr = divmod(h, 2)
            qrow = hr * 64
            pt = pt_tiles[(h, u)]
            p = ps.tile([128, CW], F32, tag=f"s{kt % 2}")
            lhsT = k_sb[qrow : qrow + 64, hp, kt * 128 : (kt + 1) * 128]
            for half in range(2):
                mm = nc.tensor.matmul(
                    p[:, half * 512 : (half + 1) * 512],
                    lhsT=lhsT,
                    rhs=q_sb[
                        qrow : qrow + 64, hp,
                        u * CW + half * 512 : u * CW + (half + 1) * 512,
                    ],
                    start=True,
                    stop=True,
                )
                if half == 1:
                    mm.ins.ldweights = False
            segs = _segments(kt, u)
            if len(segs) == 1 and segs[0][2] == "edge":
                side = segs[0][3]
                nc.scalar.activation(
                    pt[:, kt, :], p, EXP, bias=edgel_sb[:, h, side : side + 1]
                )
                return
            nc.scalar.activation(pt[:, kt, :], p, EXP)
            eng = nc.vector if kt % 2 else nc.gpsimd
            for j0, j1, kind, idx in segs:
                dst = pt[:, kt, j0 * 128 : j1 * 128]
                if kind == "mid":
                    eng.tensor_mul(
                        dst, dst, expb_sb[:, h, idx : idx + (j1 - j0), :]
                    )
                else:
                    nc.vector.tensor_scalar_mul(
                        dst, dst, edgem_sb[:, h, idx : idx + 1]
                    )

        pv_tiles = {}

        def pv_mm(h, u, kt):
            pt = pt_tiles[(h, u)]
            if kt == 0:
                pv_tiles[(h, u)] = ps.tile([DH + 1, CW], F32, tag=f"pv{u}", name=f"pv_{h}_{u}")
            # matmul output must stay within one PSUM bank: 2x512 halves
            for half in range(2):
                nc.tensor.matmul(
                    pv_tiles[(h, u)][:, half * 512 : (half + 1) * 512],
                    lhsT=v_sb[:, kt, h, :],
                    rhs=pt[:, kt, half * 512 : (half + 1) * 512],
                    start=(kt == 0),
                    stop=(kt == NKT - 1),
                )
            if kt == NKT - 1:
                # DMA (and Pool) cannot read PSUM: bounce l through SBUF
                r = 2 * h + u
                l_sb = smp.tile([1, CW], F32, tag=f"l{u}", name=f"l_{h}_{u}")
                nc.vector.tensor_copy(l_sb, pv_tiles[(h, u)][DH : DH + 1, :])
                nc.gpsimd.dma_start(out=l_dram[r : r + 1, :], in_=l_sb)

        def rec_attn(h):
            """Batched reciprocal for head h (both units) + normalize-muls
            into the paired attn tile (odd heads partition-shifted)."""
            rec_sb = smp.tile([128, 16], F32, tag="rec")
            nc.sync.dma_start(
                out=rec_sb,
                in_=bass.AP(
                    tensor=l_dram.tensor, offset=h * 2 * CW,
                    ap=[[16, 128], [1, 16]],
                ),
            )
            nc.scalar.activation(rec_sb, rec_sb, LN)
            nc.scalar.activation(rec_sb, rec_sb, EXP, scale=-1.0)
            nc.sync.dma_start(
                out=bass.AP(
                    tensor=rec_dram.tensor, offset=h * 2 * CW,
                    ap=[[16, 128], [1, 16]],
                ),
                in_=rec_sb,
            )
            hp, hr = divmod(h, 2)
            rb = hr * 64
            for u in range(2):
                bc = smp.tile([64, CW], F32, tag=f"bc{u}")
                nc.gpsimd.dma_start(
                    out=bc,
                    in_=bass.AP(
                        tensor=rec_dram.tensor, offset=h * 2 * CW + u * CW,
                        ap=[[0, 64], [1, CW]],
                    ),
                )
                nc.vector.tensor_mul(
                    attn2_sb[rb : rb + 64, hp, u * CW : (u + 1) * CW],
                    pv_tiles[(h, u)][0:DH, :],
                    bc,
                )

        # ---- emission: QKV + attention pipeline ---------------------------
        proj4(wk_sb, k_sb, 0, ("s0", "s1"))
        proj4(wq_sb, q_sb, 0, ("pv0", "pv1"))

        units = [(h, u) for h in range(HL) for u in range(2)]
        for i, (h, u) in enumerate(units):
            pt_tiles[(h, u)] = ptp.tile([128, NKT, CW], BF16, tag="pt", name=f"pt_{h}_{u}")
            for kt in range(NKT):
                sc_tile(h, u, kt)
                # interleave non-s-tag PE work so the PE is not capped by
                # the Act drain pace of the 2-tag score-psum rotation: the
                # first unit hosts the v-projection chains + ep1 q/k
                # projection passes, every later unit the previous unit's
                # P@V matmuls (keeps pt alive-generations at 2 = pool bufs)
                if i == 0:
                    v_chain(kt)
                    if kt % 4 == 3:
                        pr = kt // 4
                        proj2(
                            (wk_sb, wk_sb, wq_sb, wq_sb)[pr],
                            (k_sb, k_sb, q_sb, q_sb)[pr],
                            1, pr % 2, f"pv{pr % 2}",
                        )
                else:
                    ph, pu = units[i - 1]
                    pv_mm(ph, pu, kt)
                    if kt == NKT - 1 and pu == 1:
                        rec_attn(ph)
        for kt in range(NKT):
            pv_mm(3, 1, kt)
        rec_attn(3)

        # ---- output projection (paired heads, contraction 128) ------------
        for qi in range(N // 128):
            po = ps.tile([128, DIM], F32, tag=f"s{qi % 2}")
            for hp in range(2):
                for half in range(2):
                    nc.tensor.matmul(
                        po[:, half * 512 : (half + 1) * 512],
                        lhsT=attn2_sb[:, hp, qi * 128 : (qi + 1) * 128],
                        rhs=wo2_sb[:, hp, half * 512 : (half + 1) * 512],
                        start=(hp == 0),
                        stop=(hp == 1),
                    )
            ost = ostp.tile([128, DIM], BF16, tag="ost")
            nc.any.tensor_copy(ost, po)
            nc.sync.dma_start(out=out_p[qi * 128 : (qi + 1) * 128, :], in_=ost)


def _dedupe_ldweights(nc):
    """Drop InstLdweights whose weights AP + modes are identical to the
    previous InstLdweights in the stream (the PE array is weight-stationary;
    only another ldweights clobbers it -- no transposes in this kernel).
    Each dropped load becomes a NoOp carrying its sync waits/updates, so
    dependency semantics are unchanged; _fix_sync_waits (run after) spills
    any over-capacity wait lists."""
    for f in nc.m.functions:
        for b in f.blocks:
            prev_key = None
            out = []
            for i in b.instructions:
                if isinstance(i, mybir.InstLdweights):
                    key = (
                        str(i.ins[0]),
                        str(i.perf_mode),
                        str(i.is_transpose),
                        str(i.tile_position),
                        str(i.tile_size),
                    )
                    if key == prev_key:
                        nop = mybir.InstNoOp(
                            name=nc.get_next_instruction_name(), ins=[], outs=[]
                        )
                        nop.engine = i.engine
                        nop.sync_info = i.sync_info
                        nop.bass_nofuse = True
                        out.append(nop)
                        continue
                    prev_key = key
                out.append(i)
            b.instructions = out


def _fix_sync_waits(nc):
    """Post-schedule wait hygiene for walrus's per-struct sync-wait limits.

    1. Elide waits already implied by an earlier wait on the same engine
       (sem-ge is monotone and engines execute their instructions in order).
    2. For instructions still over their struct's wait capacity, INSERT
       NoOp wait-carriers on the same engine directly before them (strictly
       more conservative: the waits execute earlier in the same engine
       order).
    """
    import re

    _elidable = re.compile(r"^(DMASW|DMAHW|PE|DVE|Activation|Pool|SP)")
    # only instruction types whose sync_info round-trips cleanly may be
    # touched; anything else (raw-ISA customs, barriers, drains, branches)
    # is left intact and clears the elision state conservatively
    _touchable = (
        mybir.InstMatmult,
        mybir.InstNoOp,
        mybir.InstTensorTensor,
        mybir.InstTensorScalarPtr,
        mybir.InstActivation,
        mybir.InstTensorCopy,
        mybir.InstDMACopy,
        mybir.InstLdweights,
        mybir.InstMemset,
    )
    for f in nc.m.functions:
        for b in f.blocks:
            seen = {}
            for i in b.instructions:
                si = i.sync_info
                if si is None or not si.on_wait:
                    continue
                if not isinstance(i, _touchable):
                    seen.clear()
                    continue
                s = seen.setdefault(i.engine, {})
                kept = []
                for w in si.on_wait:
                    if (
                        w.wait_mode == "sem-ge-imm"
                        and _elidable.match(w.ant_name or "")
                        and s.get(w.id, -1) >= w.wait_value
                    ):
                        continue
                    kept.append(w)
                    if w.wait_mode == "sem-ge-imm" and _elidable.match(
                        w.ant_name or ""
                    ):
                        s[w.id] = w.wait_value
                if len(kept) != len(si.on_wait):
                    si.on_wait = kept

    # capacity per opcode (walrus setupSyncWait limits, found empirically:
    # Matmult fp32r=1, NoOp=1; others conservative)
    def cap_of(i):
        if isinstance(i, mybir.InstDrain):
            return 1  # spill the kernel-tail drain's wait pile onto NoOps
        if not isinstance(i, _touchable):
            return None
        return 1

    for f in nc.m.functions:
        for b in f.blocks:
            out = []
            for i in b.instructions:
                si = i.sync_info
                cap = cap_of(i)
                if si is not None and si.on_wait and cap is not None and len(
                    si.on_wait
                ) > cap:
                    waits = list(si.on_wait)
                    excess, keep = waits[:-cap], waits[-cap:]
                    while excess:
                        chunk, excess = excess[:1], excess[1:]
                        nop = mybir.InstNoOp(
                            name=nc.get_next_instruction_name(), ins=[], outs=[]
                        )
                        nop.engine = i.engine
                        nop.sync_info = mybir.SyncInfo(on_wait=chunk, on_update=[])
                        nop.bass_nofuse = True
                        out.append(nop)
                    si.on_wait = keep
                out.append(i)
            b.instructions = out


def build_program():
    global _PROGRAM
    if _PROGRAM is not None:
        return _PROGRAM
    nc = bass.Bass(trn_type="TRN2", target_bir_lowering=False, debug=False)
    xT = nc.dram_tensor("xT", [DIM, N], BF16, kind="ExternalInput").ap()
    wqT = nc.dram_tensor("wqT", [DIM, 256], BF16, kind="ExternalInput").ap()
    wkT = nc.dram_tensor("wkT", [DIM, 256], BF16, kind="ExternalInput").ap()
    wvT = nc.dram_tensor("wvT", [DIM, 256], BF16, kind="ExternalInput").ap()
    wo2T = nc.dram_tensor("wo2T", [128, 2, DIM], BF16, kind="ExternalInput").ap()
    expbT = nc.dram_tensor("expbT", [128, HL, 17, 128], BF16, kind="ExternalInput").ap()
    edge_ln = nc.dram_tensor("edge_ln", [HL, 2], F32, kind="ExternalInput").ap()
    edge_mul = nc.dram_tensor("edge_mul", [HL, 2], F32, kind="ExternalInput").ap()
    l_dram = nc.dram_tensor("l_scratch", [HL * 2, CW], F32, kind="Internal").ap()
    rec_dram = nc.dram_tensor("rec_scratch", [HL * 2, CW], F32, kind="Internal").ap()
    out_p = nc.dram_tensor("out_p", [N, DIM], BF16, kind="ExternalOutput").ap()

    with tile.TileContext(nc) as tc:
        _emit(tc, xT, wqT, wkT, wvT, wo2T, expbT, edge_ln, edge_mul, l_dram,
              rec_dram, out_p)
    _dedupe_ldweights(nc)
    _fix_sync_waits(nc)
    _PROGRAM = nc
    return nc


def make_in_maps(x, W_qkv, W_out, rel_emb):
    x = np.asarray(x, np.float32)
    W_qkv = np.asarray(W_qkv, np.float32)
    W_out = np.asarray(W_out, np.float32)
    rel_emb = np.asarray(rel_emb, np.float32)
    BF = ml_dtypes.bfloat16

    dd = np.arange(128)[:, None] - np.arange(128)[None, :]
    xTs = [np.ascontiguousarray(x[b].T).astype(BF) for b in range(x.shape[0])]
    woT = W_out.T  # [d, e]
    in_maps = []
    for c in range(8):
        b, g = c // 4, c % 4
        wq = W_qkv[g * 256 : (g + 1) * 256]
        wk = W_qkv[DIM + g * 256 : DIM + (g + 1) * 256] * np.float32(0.125)
        wv = W_qkv[2 * DIM + g * 256 : 2 * DIM + (g + 1) * 256]
        wo2 = np.ascontiguousarray(
            woT[256 * g : 256 * (g + 1)].reshape(2, 128, DIM).transpose(1, 0, 2)
        )
        bT = np.empty((HL, 17, 128, 128), np.float32)
        for hl in range(HL):
            head = 4 * g + hl
            for i in range(17):
                idx = np.clip(128 * (8 - i) + dd, -1024, 1024) + 1024
                bT[hl, i] = np.exp(rel_emb[idx, head])
        eln = np.stack(
            [rel_emb[0, 4 * g : 4 * g + 4], rel_emb[2048, 4 * g : 4 * g + 4]],
            axis=1,
        )
        in_maps.append(
            {
                "xT": xTs[b],
                "wqT": np.ascontiguousarray(wq.T).astype(BF),
                "wkT": np.ascontiguousarray(wk.T).astype(BF),
                "wvT": np.ascontiguousarray(wv.T).astype(BF),
                "wo2T": wo2.astype(BF),
                "expbT": np.ascontiguousarray(
                    bT.transpose(2, 0, 1, 3)
                ).astype(BF),
                "edge_ln": np.ascontiguousarray(eln, np.float32),
                "edge_mul": np.exp(eln).astype(np.float32),
            }
        )
    return in_maps


def combine_outputs(results, b_out):
    b_out = np.asarray(b_out, np.float32)
    out = np.empty((2, N, DIM), np.float32)
    for b in range(2):
        acc = results[4 * b]["out_p"].astype(np.float32)
        for g in range(1, 4):
            acc = acc + results[4 * b + g]["out_p"].astype(np.float32)
        out[b] = acc + b_out[None, :]
    return out


def kernel(x, W_qkv, W_out, b_out, rel_emb):
    global LAST_RESULTS
    nc = build_program()
    in_maps = make_in_maps(x, W_qkv, W_out, rel_emb)
    LAST_RESULTS = run_bass_kernel_spmd(nc, in_maps, list(range(8)))
    return combine_outputs(LAST_RESULTS.results, b_out)
